# revision 31
# baseline (speedup 1.0000x reference)
"""Trainium2 Bass kernel for nn_AsRRN (all-pairs GNN message passing).

Strategy (8 NeuronCores, SPMD):
  - Node axis 513 padded to 520 = 8*65; core r owns rows [65r, 65r+65).
  - Per step, each core computes its 65-row shard of the masked message
    sum and state update; only mh = states @ Wf_h (bf16, 256 x 66 per
    core incl. one zero pad column) is all-gathered per step.
  - Message inner loop, msg-dim k on partitions (2 tiles of 128), all
    528 j's (8 ranks x 66) on the free axis; the two passes pipeline
    across engines per (i, k-tile):
      A (ScalarE): t = relu(mhT + mc'_i)     (ACTIVATE, per-partition bias)
      B (VectorE): (t + 0) * mask_i with accum_out = sum_j
                   (scalar_tensor_tensor: fused mask-multiply + reduction)
    Mask rows are replicated across all 128 partitions once into a
    persistent SBUF blob via PE rank-1 broadcast matmuls + DVE casts
    (the mask is step-invariant). States update uses 18 N=512 swapped
    matmuls (row-major, bg folded in via a ones-row) + PE transposes
    back to T-layout. Matmul phases are kept strictly serialized away
    from the message loop: concurrent PE SBUF streaming measurably
    inflates DVE/ACT op latencies (~20%).

Toolchain constraint driving the structure: this walrus encodes at most
ONE sync-wait per TPB instruction. Hence: startup loads ride SWDGE
queues, the 8 per-step HWDGE DMAs (4x AG-in + 4x regather) are each the
first on their DMA proc, every cross-engine DMA-queue dependency is
pre-absorbed by a sacrificial 1-element copy or dummy matmul, and bias
adds are folded into matmuls or per-partition ACT/DVE operands.
"""

import sys

sys.path.insert(0, "/opt/trn_rl_repo")

import numpy as np
import ml_dtypes

import concourse.bass as bass
import concourse.bacc as bacc
import concourse.mybir as mybir
import concourse.tile as tile
from concourse.bass_utils import run_bass_kernel_spmd

NCORES = 8
H = 1024
MSG = 256
NP1 = 513
PAD = 520            # 8 * 65 (row padding)
SH = PAD // NCORES   # 65 rows per core
MB = SH + 1          # 66: per-rank j block (65 rows + 1 zero pad col)
JP = NCORES * MB     # 528: padded all-pairs j extent
KT = MSG // 128      # 2 k-tiles
HT = H // 128        # 8 h-tiles
GT = (2 * H + MSG) // 128  # 18 contraction tiles for Wg
ANS_LOCAL = 512 - 7 * SH   # local row of global node 512 on core 7 (=57)

# wblob column layout (bf16 units)
WFC = 0
WFH = WFC + HT * MSG          # 2048
WG = WFH + HT * MSG           # 4096
WO = WG + GT * H              # 22528
INIT = WO + HT * 5            # 22568
BF = INIT + HT * SH           # 23088 (f32 (128,2) -> 4 cols, even offset)
BGC = BF + 2 * KT             # 23092 (f32 (128,8) -> 16 cols, even offset)
ONES128 = BGC + 2 * HT        # 23108 (all partitions = 1.0)
ONES65 = ONES128 + 128        # 23236 (p0 only)
BGROW = ONES65 + SH           # 23301 (p0 only, bf16 bg row)
BO = BGROW + H + 1            # 24326 (p0 only, f32 (1,5) -> 10 cols, even)
IDENT = BO + 10               # 24336 (128x128 bf16 identity for PE transpose)
NCOL = IDENT + 128            # 24464

# mask staging layout: row i lives at partition 32*(i%3), col block i//3
MS_ROWS = 96
MS_BLKS = -(-SH // 3)         # 22
MS_COLS = MS_BLKS * (NCORES * MB)

F32 = mybir.dt.float32
BF16 = mybir.dt.bfloat16
ALU = mybir.AluOpType
ACTF = mybir.ActivationFunctionType
BFNP = ml_dtypes.bfloat16

# wblob is split into chunks loaded by separate SWDGE DMAs (parallel
# queues); chunk boundaries in bf16 columns
WCHUNKS = [0, WG, WG + 9216, WO, NCOL]


def build_nc(n_steps: int) -> bass.Bass:
    nc = bacc.Bacc(
        "TRN2", target_bir_lowering=False, debug=False, num_devices=NCORES
    )

    p_wblob = nc.declare_dram_parameter("wblob", [128, NCOL], BF16, False)
    p_maskb = nc.declare_dram_parameter("maskb", [MS_ROWS, MS_COLS], BF16, False)
    p_out = nc.declare_dram_parameter("outb", [SH, H], F32, True)
    p_lg = nc.declare_dram_parameter("out_logits", [1, 5], F32, True)

    with tile.TileContext(nc) as tc:
        with (
            tc.tile_pool(name="const", bufs=1) as cpool,
            tc.tile_pool(name="state", bufs=4) as spool,
            tc.tile_pool(name="mh", bufs=4) as mhpool,
            tc.tile_pool(name="work", bufs=4) as wpool,
            tc.tile_pool(name="psA", bufs=2, space="PSUM") as psA,
            tc.tile_pool(name="psG", bufs=2, space="PSUM") as psG,
            tc.tile_pool(name="psF", bufs=2, space="PSUM") as psF,
            tc.tile_pool(name="dram", bufs=4, space="DRAM") as dpool,
        ):
            # ---------------- startup loads (SWDGE queues) ----------------
            # mask staging first and chunked: the blob broadcasts (and with
            # them step 0's message loop) are gated on it
            ms = cpool.tile([MS_ROWS, MS_COLS], BF16)
            msc = MS_COLS // 6
            for c in range(6):
                hi = MS_COLS if c == 5 else (c + 1) * msc
                nc.gpsimd.dma_start(ms[:, c * msc : hi], p_maskb[:, c * msc : hi])
            wb = cpool.tile([128, NCOL], BF16)
            for c0, c1 in zip(WCHUNKS[:-1], WCHUNKS[1:]):
                nc.gpsimd.dma_start(wb[:, c0:c1], p_wblob[:, c0:c1])
            st0 = cpool.tile([128, HT * SH], BF16)
            nc.gpsimd.dma_start(st0[:], p_wblob[:, INIT : INIT + HT * SH])

            bf_v = wb[:, BF : BF + 2 * KT].bitcast(F32)      # (128, KT)
            bgc_v = wb[:, BGC : BGC + 2 * HT].bitcast(F32)   # (128, HT)
            bo_v = wb[0:1, BO : BO + 10].bitcast(F32)        # (1, 5)

            # ---------------- wait absorbers ----------------
            # PE: one dummy matmul per SWDGE-loaded tile/chunk so real
            # matmuls carry at most one unobserved wait each
            for c0, _ in zip(WCHUNKS[:-1], WCHUNKS[1:]):
                pd = psA.tile([1, 1], F32, tag="psA")
                nc.tensor.matmul(
                    pd[:], lhsT=wb[:, c0 : c0 + 1], rhs=wb[:, c0 : c0 + 1],
                    start=True, stop=True,
                )
            pd = psA.tile([1, 1], F32, tag="psA")
            nc.tensor.matmul(
                pd[:], lhsT=st0[:, 0:1], rhs=st0[:, 0:1], start=True, stop=True
            )
            pd = psA.tile([1, 1], F32, tag="psA")
            nc.tensor.matmul(
                pd[:], lhsT=ms[0:96, 0:1], rhs=ms[0:96, 0:1], start=True, stop=True
            )

            # DVE + ACT: 1-element copies observing the SWDGE queues
            absorb = cpool.tile([1, 16], F32)
            nc.vector.tensor_copy(absorb[0:1, 0:1], wb[0:1, BF : BF + 1])
            nc.scalar.copy(absorb[0:1, 1:2], wb[0:1, BGC : BGC + 1])

            # preload exp table set early (softmax at the very end)
            dummy = cpool.tile([1, 1], F32)
            nc.gpsimd.memset(dummy[:], 0.0)
            nc.scalar.activation(dummy[:], dummy[:], ACTF.Exp)

            def dep_nop(eng, producers):
                n = eng.nop(hint="wait_absorb")
                for p in producers:
                    bass._add_dep_helper(n.ins, p.ins, sync=True, reason="absorb")
                return n

            blob = cpool.tile([128, SH * JP], BF16)
            blob_absorb = cpool.tile([1, 2 * SH], F32)
            statesT = st0

            for s in range(n_steps):
                last = s == n_steps - 1

                # ---- mc' = states @ Wf_c + bf (fp32), mh local (bf16) ----
                mcp = wpool.tile([128, KT * SH], F32, tag="mcp")
                mhl = wpool.tile([128, KT * MB], BF16, tag="mhl")
                nc.vector.memset(mhl[:, SH :: MB], 0.0)  # zero the pad cols
                mhl_insts = []
                for kt in range(KT):
                    ps = psA.tile([128, SH], F32, tag="psA")
                    for ht in range(HT):
                        nc.tensor.matmul(
                            ps[:],
                            lhsT=wb[
                                :,
                                WFC + ht * MSG + kt * 128 : WFC
                                + ht * MSG
                                + kt * 128
                                + 128,
                            ],
                            rhs=statesT[:, ht * SH : (ht + 1) * SH],
                            start=(ht == 0),
                            stop=(ht == HT - 1),
                        )
                    nc.vector.tensor_scalar(
                        mcp[:, kt * SH : (kt + 1) * SH],
                        ps[:],
                        bf_v[:, kt : kt + 1],
                        None,
                        ALU.add,
                    )
                    ps2 = psA.tile([128, SH], F32, tag="psA")
                    for ht in range(HT):
                        nc.tensor.matmul(
                            ps2[:],
                            lhsT=wb[
                                :,
                                WFH + ht * MSG + kt * 128 : WFH
                                + ht * MSG
                                + kt * 128
                                + 128,
                            ],
                            rhs=statesT[:, ht * SH : (ht + 1) * SH],
                            start=(ht == 0),
                            stop=(ht == HT - 1),
                        )
                    mhl_insts.append(
                        nc.vector.tensor_copy(
                            mhl[:, kt * MB : kt * MB + SH], ps2[:]
                        )
                    )

                # ---- AllGather mh ----
                cc_in = dpool.tile([128, KT * MB], BF16, tag="ccin")
                nc.sync.dma_start(cc_in[:], mhl[:])
                cc_out = dpool.tile([NCORES * 128, KT * MB], BF16, tag="ccout")
                cc_inst = nc.gpsimd.collective_compute(
                    "AllGather",
                    ALU.bypass,
                    replica_groups=[list(range(NCORES))],
                    ins=[cc_in[:].opt()],
                    outs=[cc_out[:].opt()],
                )
                # regather to mhT[p, r*132 + kt*66 + m] in ONE DMA
                mhT = mhpool.tile([128, NCORES * KT * MB], BF16, tag="mhT")
                nc.sync.dma_start(
                    mhT[:].rearrange("p (r c) -> p r c", r=NCORES),
                    cc_out[:].rearrange("(r p) c -> p r c", r=NCORES),
                )
                nc.vector.tensor_copy(absorb[0:1, 2 + s : 3 + s], mhT[0:1, 0:1])

                if s == 0:
                    # ---- mask blob build: PE rank-1 broadcast + ACT copy ----
                    # emitted after the AG issue so step-0 mc/mh matmuls keep
                    # scheduler priority; rows pipeline with the i-loop below
                    for i in range(SH):
                        bp = 32 * (i % 3)
                        blk = (i // 3) * JP
                        for hf in range(2):
                            pb = psF.tile([128, JP // 2], F32, tag="psF")
                            nc.tensor.matmul(
                                pb[:],
                                lhsT=wb[bp : bp + 1, ONES128 : ONES128 + 128],
                                rhs=ms[
                                    bp : bp + 1,
                                    blk + hf * (JP // 2) : blk + (hf + 1) * (JP // 2),
                                ],
                                start=True,
                                stop=True,
                            )
                            nc.vector.tensor_copy(
                                blob[
                                    :,
                                    i * JP + hf * (JP // 2) : i * JP
                                    + (hf + 1) * (JP // 2),
                                ],
                                pb[:],
                            )
                        nc.vector.tensor_copy(
                            blob_absorb[0:1, 2 * i : 2 * i + 2],
                            blob[0:1, i * JP + JP // 2 - 1 : i * JP + JP // 2 + 1],
                        )

                # ---- message inner loop ----
                # A (relu + per-partition bias) on the ScalarEngine, B (mask
                # multiply + j-sum) on the VectorEngine; the two pipeline
                msgT = wpool.tile([128, KT * SH], F32, tag="msgT")
                mhT_r = mhT[:].rearrange("p (r c) -> p r c", r=NCORES)
                for i in range(SH):
                    for kt in range(KT):
                        t_t = wpool.tile([128, JP], BF16, tag="t")
                        nc.scalar.activation(
                            t_t[:].rearrange("p (r m) -> p r m", r=NCORES),
                            mhT_r[:, :, kt * MB : (kt + 1) * MB],
                            ACTF.Relu,
                            bias=mcp[:, kt * SH + i : kt * SH + i + 1],
                        )
                        scr = wpool.tile([128, JP], BF16, tag="scr")
                        nc.vector.scalar_tensor_tensor(
                            out=scr[:],
                            in0=t_t[:],
                            scalar=0.0,
                            in1=blob[:, i * JP : (i + 1) * JP],
                            op0=ALU.add,
                            op1=ALU.mult,
                            accum_out=msgT[:, kt * SH + i : kt * SH + i + 1],
                        )

                msgTb = wpool.tile([128, KT * SH], BF16, tag="msgTb")
                nc.vector.tensor_copy(msgTb[:], msgT[:])

                # ---- states = relu(g_in @ Wg + bg), T-layout ----
                rhs_tiles = (
                    [statesT[:, t * SH : (t + 1) * SH] for t in range(HT)]
                    + [
                        wb[:, INIT + t * SH : INIT + (t + 1) * SH]
                        for t in range(HT)
                    ]
                    + [msgTb[:, t * SH : (t + 1) * SH] for t in range(KT)]
                )
                # row-major states via swapped matmul (18 big-N matmuls
                # instead of 144 small ones); bg folded in via a ones-row
                strow = spool.tile([SH, H], BF16, tag="strow")
                if last:
                    fin = wpool.tile([SH, H], F32, tag="fin")
                for nt in range(2):
                    psr = psG.tile([SH, 512], F32, tag="psG")
                    for g in range(GT):
                        nc.tensor.matmul(
                            psr[:],
                            lhsT=rhs_tiles[g],
                            rhs=wb[
                                :, WG + g * H + nt * 512 : WG + g * H + nt * 512 + 512
                            ],
                            start=(g == 0),
                            stop=False,
                        )
                    nc.tensor.matmul(
                        psr[:],
                        lhsT=wb[0:1, ONES65 : ONES65 + SH],
                        rhs=wb[0:1, BGROW + nt * 512 : BGROW + nt * 512 + 512],
                        start=False,
                        stop=True,
                    )
                    nc.vector.tensor_scalar(
                        strow[:, nt * 512 : (nt + 1) * 512], psr[:], 0.0, None, ALU.max
                    )
                    if last:
                        nc.vector.tensor_scalar(
                            fin[0:SH, nt * 512 : (nt + 1) * 512],
                            psr[:],
                            0.0,
                            None,
                            ALU.max,
                        )
                # back to T-layout via PE transpose
                statesT_new = spool.tile([128, HT * SH], BF16, tag="stT")
                for ht in range(HT):
                    pst = psA.tile([128, SH], BF16, tag="psT")
                    nc.tensor.transpose(
                        pst[:],
                        strow[0:SH, ht * 128 : (ht + 1) * 128],
                        wb[0:SH, IDENT : IDENT + SH],
                    )
                    nc.vector.tensor_copy(
                        statesT_new[:, ht * SH : (ht + 1) * SH], pst[:]
                    )

                if last:
                    fin = wpool.tile([SH, H], F32, tag="fin")
                    # row-major fp32 states; bg folded in via a ones-row
                    for nt in range(2):
                        psf = psF.tile([SH, 512], F32, tag="psF")
                        for g in range(GT):
                            nc.tensor.matmul(
                                psf[:],
                                lhsT=rhs_tiles[g],
                                rhs=wb[
                                    :,
                                    WG + g * H + nt * 512 : WG + g * H + nt * 512 + 512,
                                ],
                                start=(g == 0),
                                stop=False,
                            )
                        nc.tensor.matmul(
                            psf[:],
                            lhsT=wb[0:1, ONES65 : ONES65 + SH],
                            rhs=wb[0:1, BGROW + nt * 512 : BGROW + nt * 512 + 512],
                            start=False,
                            stop=True,
                        )
                        nc.vector.tensor_scalar(
                            fin[0:SH, nt * 512 : (nt + 1) * 512],
                            psf[:],
                            0.0,
                            None,
                            ALU.max,
                        )

                    # logits from local row ANS_LOCAL (core 7's is the answer)
                    psl = psA.tile([1, 5], F32, tag="psA")
                    for ht in range(HT):
                        nc.tensor.matmul(
                            psl[:],
                            lhsT=statesT_new[
                                :, ht * SH + ANS_LOCAL : ht * SH + ANS_LOCAL + 1
                            ],
                            rhs=wb[:, WO + ht * 5 : WO + (ht + 1) * 5],
                            start=(ht == 0),
                            stop=(ht == HT - 1),
                        )
                    xlg = wpool.tile([1, 5], F32, tag="lg")
                    nc.vector.tensor_tensor(xlg[:], psl[:], bo_v, ALU.add)
                    mx = wpool.tile([1, 1], F32, tag="lg1")
                    nc.vector.tensor_reduce(
                        mx[:], xlg[:], mybir.AxisListType.X, ALU.max
                    )
                    nmx = wpool.tile([1, 1], F32, tag="lg2")
                    nc.vector.tensor_scalar(nmx[:], mx[:], -1.0, None, ALU.mult)
                    elg = wpool.tile([1, 5], F32, tag="lg3")
                    nc.scalar.activation(elg[:], xlg[:], ACTF.Exp, bias=nmx[:, 0:1])
                    ssum = wpool.tile([1, 1], F32, tag="lg4")
                    nc.vector.tensor_reduce(
                        ssum[:], elg[:], mybir.AxisListType.X, ALU.add
                    )
                    rcp = wpool.tile([1, 1], F32, tag="lg5")
                    nc.vector.reciprocal(rcp[:], ssum[:])
                    lgo = wpool.tile([1, 5], F32, tag="lg6")
                    nc.vector.tensor_scalar(
                        lgo[:], elg[:], rcp[:, 0:1], None, ALU.mult
                    )
                    nc.gpsimd.dma_start(p_out[:], fin[:])
                    nc.gpsimd.dma_start(p_lg[:], lgo[:])

                statesT = statesT_new

    nc.finalize()
    return nc


_NC_CACHE: dict[int, bass.Bass] = {}


def _f32_as_bf16pair(a):
    # reinterpret a float32 array as pairs of bf16-sized uint16 lanes
    a = np.ascontiguousarray(a.astype(np.float32))
    return a.view(np.uint16).view(BFNP)


def _prep_wblob(Wfc, Wfh, Wg, Wo, bf, bg, bo, initT):
    wb = np.zeros((128, NCOL), BFNP)
    wb[:, WFC : WFC + HT * MSG] = (
        Wfc.reshape(HT, 128, MSG).transpose(1, 0, 2).reshape(128, HT * MSG)
    ).astype(BFNP)
    wb[:, WFH : WFH + HT * MSG] = (
        Wfh.reshape(HT, 128, MSG).transpose(1, 0, 2).reshape(128, HT * MSG)
    ).astype(BFNP)
    wb[:, WG : WG + GT * H] = (
        Wg.reshape(GT, 128, H).transpose(1, 0, 2).reshape(128, GT * H)
    ).astype(BFNP)
    wb[:, WO : WO + HT * 5] = (
        Wo.reshape(HT, 128, 5).transpose(1, 0, 2).reshape(128, HT * 5)
    ).astype(BFNP)
    wb[:, INIT : INIT + HT * SH] = initT.astype(BFNP)
    wb[:, BF : BF + 2 * KT] = _f32_as_bf16pair(bf.reshape(KT, 128).T.copy())
    wb[:, BGC : BGC + 2 * HT] = _f32_as_bf16pair(bg.reshape(HT, 128).T.copy())
    wb[:, ONES128 : ONES128 + 128] = np.ones((128, 128), BFNP)
    wb[0, ONES65 : ONES65 + SH] = np.ones(SH, BFNP)
    wb[0, BGROW : BGROW + H] = bg.astype(BFNP)
    wb[0, BO : BO + 10] = _f32_as_bf16pair(bo.reshape(1, 5))[0]
    wb[:, IDENT : IDENT + 128] = np.eye(128, dtype=np.float32).astype(BFNP)
    return wb


def kernel(pooled_output, super_node, mask, Wf, bf, Wg, bg, Wo, bo, n_steps):
    n_steps = int(n_steps)
    pooled_output = np.asarray(pooled_output, np.float32)
    super_node = np.asarray(super_node, np.float32)
    mask = np.asarray(mask, np.float32)
    Wf = np.asarray(Wf, np.float32)
    bf = np.asarray(bf, np.float32)
    Wg = np.asarray(Wg, np.float32)
    bg = np.asarray(bg, np.float32)
    Wo = np.asarray(Wo, np.float32)
    bo = np.asarray(bo, np.float32)

    initial = np.concatenate([super_node, pooled_output], axis=0)  # (513, H)
    init_pad = np.zeros((PAD, H), np.float32)
    init_pad[:NP1] = initial
    mask2 = mask.reshape(NP1, NP1)
    mask_pad = np.zeros((PAD, PAD), np.float32)
    mask_pad[:NP1, :NP1] = mask2

    Wfc, Wfh = np.ascontiguousarray(Wf[:H]), np.ascontiguousarray(Wf[H:])

    in_maps = []
    for r in range(NCORES):
        sl = slice(r * SH, (r + 1) * SH)
        initT_r = np.ascontiguousarray(init_pad[sl].T)  # (H, 65)
        initT_r = (
            initT_r.reshape(HT, 128, SH).transpose(1, 0, 2).reshape(128, HT * SH)
        )  # [p, t*65+i]
        wb = _prep_wblob(Wfc, Wfh, Wg, Wo, bf, bg, bo, initT_r)
        # mask rows in blob j-order: col (rr*66 + m) = mask_pad[gi, 65*rr+m]
        mrows = np.zeros((SH, NCORES * MB), np.float32)
        msk = mask_pad[sl, :]  # (65, 520)
        for rr in range(NCORES):
            mrows[:, rr * MB : rr * MB + SH] = msk[:, rr * SH : (rr + 1) * SH]
        msb = np.zeros((MS_ROWS, MS_COLS), BFNP)
        JPc = NCORES * MB
        for i in range(SH):
            msb[32 * (i % 3), (i // 3) * JPc : (i // 3 + 1) * JPc] = mrows[i].astype(
                BFNP
            )
        in_maps.append(dict(wblob=wb, maskb=msb))

    if n_steps not in _NC_CACHE:
        _NC_CACHE[n_steps] = build_nc(n_steps)
    nc = _NC_CACHE[n_steps]

    res = run_bass_kernel_spmd(nc, in_maps, core_ids=list(range(NCORES)))
    global LAST_RESULT
    LAST_RESULT = res
    results = res.results
    states = np.concatenate(
        [np.asarray(results[r]["outb"], np.float32) for r in range(NCORES)],
        axis=0,
    )[:NP1]
    logits = np.asarray(results[7]["out_logits"], np.float32).copy()
    return logits, states


LAST_RESULT = None


# revision 32
# speedup vs baseline: 1.0252x; 1.0252x over previous
"""Trainium2 Bass kernel for nn_AsRRN (all-pairs GNN message passing).

Strategy (8 NeuronCores, SPMD):
  - Node axis 513 padded to 520 = 8*65; core r owns rows [65r, 65r+65).
  - Per step, each core computes its 65-row shard of the masked message
    sum and state update; only mh = states @ Wf_h (bf16, 256 x 66 per
    core incl. one zero pad column) is all-gathered per step.
  - Message inner loop, msg-dim k on partitions (2 tiles of 128), all
    528 j's (8 ranks x 66) on the free axis; the two passes pipeline
    across engines per (i, k-tile):
      A (ScalarE): t = relu(mhT + mc'_i)     (ACTIVATE, per-partition bias)
      B (VectorE): (t + 0) * mask_i with accum_out = sum_j
                   (scalar_tensor_tensor: fused mask-multiply + reduction)
    Mask rows are replicated across all 128 partitions once into a
    persistent SBUF blob via PE rank-1 broadcast matmuls + DVE casts
    (the mask is step-invariant). States update uses 18 N=512 swapped
    matmuls (row-major, bg folded in via a ones-row) + PE transposes
    back to T-layout. Matmul phases are kept strictly serialized away
    from the message loop: concurrent PE SBUF streaming measurably
    inflates DVE/ACT op latencies (~20%).

Toolchain constraint driving the structure: this walrus encodes at most
ONE sync-wait per TPB instruction. Hence: startup loads ride SWDGE
queues, the 8 per-step HWDGE DMAs (4x AG-in + 4x regather) are each the
first on their DMA proc, every cross-engine DMA-queue dependency is
pre-absorbed by a sacrificial 1-element copy or dummy matmul, and bias
adds are folded into matmuls or per-partition ACT/DVE operands.
"""

import sys

sys.path.insert(0, "/opt/trn_rl_repo")

import numpy as np
import ml_dtypes

import concourse.bass as bass
import concourse.bacc as bacc
import concourse.mybir as mybir
import concourse.tile as tile
from concourse.bass_utils import run_bass_kernel_spmd

NCORES = 8
H = 1024
MSG = 256
NP1 = 513
PAD = 520            # 8 * 65 (row padding)
SH = PAD // NCORES   # 65 rows per core
MB = SH + 1          # 66: per-rank j block (65 rows + 1 zero pad col)
JP = NCORES * MB     # 528: padded all-pairs j extent
KT = MSG // 128      # 2 k-tiles
HT = H // 128        # 8 h-tiles
GT = (2 * H + MSG) // 128  # 18 contraction tiles for Wg
ANS_LOCAL = 512 - 7 * SH   # local row of global node 512 on core 7 (=57)

# wblob column layout (bf16 units)
WFC = 0
WFH = WFC + HT * MSG          # 2048
WG = WFH + HT * MSG           # 4096
WO = WG + GT * H              # 22528
INIT = WO + HT * 5            # 22568
BF = INIT + HT * SH           # 23088 (f32 (128,2) -> 4 cols, even offset)
BGC = BF + 2 * KT             # 23092 (f32 (128,8) -> 16 cols, even offset)
ONES128 = BGC + 2 * HT        # 23108 (all partitions = 1.0)
ONES65 = ONES128 + 128        # 23236 (p0 only)
BGROW = ONES65 + SH           # 23301 (p0 only, bf16 bg row)
BO = BGROW + H + 1            # 24326 (p0 only, f32 (1,5) -> 10 cols, even)
IDENT = BO + 10               # 24336 (128x128 bf16 identity for PE transpose)
NCOL = IDENT + 128            # 24464

# mask staging layout: row i lives at partition 32*(i%3), col block i//3
MS_ROWS = 96
MS_BLKS = -(-SH // 3)         # 22
MS_COLS = MS_BLKS * (NCORES * MB)

F32 = mybir.dt.float32
BF16 = mybir.dt.bfloat16
ALU = mybir.AluOpType
ACTF = mybir.ActivationFunctionType
BFNP = ml_dtypes.bfloat16

# wblob is split into chunks loaded by separate SWDGE DMAs (parallel
# queues); chunk boundaries in bf16 columns
WCHUNKS = [0, WG, WG + 9216, WO, NCOL]


def build_nc(n_steps: int) -> bass.Bass:
    nc = bacc.Bacc(
        "TRN2", target_bir_lowering=False, debug=False, num_devices=NCORES
    )

    p_wblob = nc.declare_dram_parameter("wblob", [128, NCOL], BF16, False)
    p_maskb = nc.declare_dram_parameter("maskb", [MS_ROWS, MS_COLS], BF16, False)
    p_out = nc.declare_dram_parameter("outb", [SH, H], F32, True)
    p_lg = nc.declare_dram_parameter("out_logits", [1, 5], F32, True)

    with tile.TileContext(nc) as tc:
        with (
            tc.tile_pool(name="const", bufs=1) as cpool,
            tc.tile_pool(name="state", bufs=4) as spool,
            tc.tile_pool(name="mh", bufs=4) as mhpool,
            tc.tile_pool(name="work", bufs=4) as wpool,
            tc.tile_pool(name="psA", bufs=2, space="PSUM") as psA,
            tc.tile_pool(name="psG", bufs=2, space="PSUM") as psG,
            tc.tile_pool(name="psF", bufs=2, space="PSUM") as psF,
            tc.tile_pool(name="dram", bufs=4, space="DRAM") as dpool,
        ):
            # ---------------- startup loads (SWDGE queues) ----------------
            # mask staging first and chunked: the blob broadcasts (and with
            # them step 0's message loop) are gated on it
            ms = cpool.tile([MS_ROWS, MS_COLS], BF16)
            msc = MS_COLS // 6
            for c in range(6):
                hi = MS_COLS if c == 5 else (c + 1) * msc
                nc.gpsimd.dma_start(ms[:, c * msc : hi], p_maskb[:, c * msc : hi])
            wb = cpool.tile([128, NCOL], BF16)
            for c0, c1 in zip(WCHUNKS[:-1], WCHUNKS[1:]):
                nc.gpsimd.dma_start(wb[:, c0:c1], p_wblob[:, c0:c1])
            st0 = cpool.tile([128, HT * SH], BF16)
            nc.gpsimd.dma_start(st0[:], p_wblob[:, INIT : INIT + HT * SH])

            bf_v = wb[:, BF : BF + 2 * KT].bitcast(F32)      # (128, KT)
            bgc_v = wb[:, BGC : BGC + 2 * HT].bitcast(F32)   # (128, HT)
            bo_v = wb[0:1, BO : BO + 10].bitcast(F32)        # (1, 5)

            # ---------------- wait absorbers ----------------
            # PE: one dummy matmul per SWDGE-loaded tile/chunk so real
            # matmuls carry at most one unobserved wait each
            for c0, _ in zip(WCHUNKS[:-1], WCHUNKS[1:]):
                pd = psA.tile([1, 1], F32, tag="psA")
                nc.tensor.matmul(
                    pd[:], lhsT=wb[:, c0 : c0 + 1], rhs=wb[:, c0 : c0 + 1],
                    start=True, stop=True,
                )
            pd = psA.tile([1, 1], F32, tag="psA")
            nc.tensor.matmul(
                pd[:], lhsT=st0[:, 0:1], rhs=st0[:, 0:1], start=True, stop=True
            )
            pd = psA.tile([1, 1], F32, tag="psA")
            nc.tensor.matmul(
                pd[:], lhsT=ms[0:96, 0:1], rhs=ms[0:96, 0:1], start=True, stop=True
            )

            # DVE + ACT: 1-element copies observing the SWDGE queues
            absorb = cpool.tile([1, 16], F32)
            nc.vector.tensor_copy(absorb[0:1, 0:1], wb[0:1, BF : BF + 1])
            nc.scalar.copy(absorb[0:1, 1:2], wb[0:1, BGC : BGC + 1])

            # preload exp table set early (softmax at the very end)
            dummy = cpool.tile([1, 1], F32)
            nc.gpsimd.memset(dummy[:], 0.0)
            nc.scalar.activation(dummy[:], dummy[:], ACTF.Exp)

            def dep_nop(eng, producers):
                n = eng.nop(hint="wait_absorb")
                for p in producers:
                    bass._add_dep_helper(n.ins, p.ins, sync=True, reason="absorb")
                return n

            blob = cpool.tile([128, SH * JP], BF16)
            blob_absorb = cpool.tile([1, 2 * SH], F32)
            statesT = st0

            for s in range(n_steps):
                last = s == n_steps - 1

                # ---- mc' = states @ Wf_c + bf (fp32), mh local (bf16) ----
                mcp = wpool.tile([128, KT * SH], F32, tag="mcp")
                mhl = wpool.tile([128, KT * MB], BF16, tag="mhl")
                nc.vector.memset(mhl[:, SH :: MB], 0.0)  # zero the pad cols
                mhl_insts = []
                for kt in range(KT):
                    ps = psA.tile([128, SH], F32, tag="psA")
                    for ht in range(HT):
                        nc.tensor.matmul(
                            ps[:],
                            lhsT=wb[
                                :,
                                WFC + ht * MSG + kt * 128 : WFC
                                + ht * MSG
                                + kt * 128
                                + 128,
                            ],
                            rhs=statesT[:, ht * SH : (ht + 1) * SH],
                            start=(ht == 0),
                            stop=(ht == HT - 1),
                        )
                    nc.vector.tensor_scalar(
                        mcp[:, kt * SH : (kt + 1) * SH],
                        ps[:],
                        bf_v[:, kt : kt + 1],
                        None,
                        ALU.add,
                    )
                    ps2 = psA.tile([128, SH], F32, tag="psA")
                    for ht in range(HT):
                        nc.tensor.matmul(
                            ps2[:],
                            lhsT=wb[
                                :,
                                WFH + ht * MSG + kt * 128 : WFH
                                + ht * MSG
                                + kt * 128
                                + 128,
                            ],
                            rhs=statesT[:, ht * SH : (ht + 1) * SH],
                            start=(ht == 0),
                            stop=(ht == HT - 1),
                        )
                    mhl_insts.append(
                        nc.vector.tensor_copy(
                            mhl[:, kt * MB : kt * MB + SH], ps2[:]
                        )
                    )

                # ---- AllGather mh ----
                cc_in = dpool.tile([128, KT * MB], BF16, tag="ccin")
                nc.sync.dma_start(cc_in[:], mhl[:])
                cc_out = dpool.tile([NCORES * 128, KT * MB], BF16, tag="ccout")
                cc_inst = nc.gpsimd.collective_compute(
                    "AllGather",
                    ALU.bypass,
                    replica_groups=[list(range(NCORES))],
                    ins=[cc_in[:].opt()],
                    outs=[cc_out[:].opt()],
                )
                # regather to mhT[p, r*132 + kt*66 + m] in ONE DMA
                mhT = mhpool.tile([128, NCORES * KT * MB], BF16, tag="mhT")
                nc.sync.dma_start(
                    mhT[:].rearrange("p (r c) -> p r c", r=NCORES),
                    cc_out[:].rearrange("(r p) c -> p r c", r=NCORES),
                )
                nc.vector.tensor_copy(absorb[0:1, 2 + s : 3 + s], mhT[0:1, 0:1])

                if s == 0:
                    # ---- mask blob build: PE rank-1 broadcast + ACT copy ----
                    # emitted after the AG issue so step-0 mc/mh matmuls keep
                    # scheduler priority; rows pipeline with the i-loop below
                    for i in range(SH):
                        bp = 32 * (i % 3)
                        blk = (i // 3) * JP
                        for hf in range(2):
                            pb = psF.tile([128, JP // 2], F32, tag="psF")
                            nc.tensor.matmul(
                                pb[:],
                                lhsT=wb[bp : bp + 1, ONES128 : ONES128 + 128],
                                rhs=ms[
                                    bp : bp + 1,
                                    blk + hf * (JP // 2) : blk + (hf + 1) * (JP // 2),
                                ],
                                start=True,
                                stop=True,
                            )
                            nc.vector.tensor_copy(
                                blob[
                                    :,
                                    i * JP + hf * (JP // 2) : i * JP
                                    + (hf + 1) * (JP // 2),
                                ],
                                pb[:],
                            )
                        nc.vector.tensor_copy(
                            blob_absorb[0:1, 2 * i : 2 * i + 2],
                            blob[0:1, i * JP + JP // 2 - 1 : i * JP + JP // 2 + 1],
                        )

                # ---- message inner loop ----
                # A (relu + per-partition bias) on the ScalarEngine, B (mask
                # multiply + j-sum) on the VectorEngine; the two pipeline
                msgT = wpool.tile([128, KT * SH], F32, tag="msgT")
                mhT_r = mhT[:].rearrange("p (r c) -> p r c", r=NCORES)
                late_b = None
                for i in range(SH):
                    for kt in range(KT):
                        t_t = wpool.tile([128, JP], BF16, tag="t")
                        nc.scalar.activation(
                            t_t[:].rearrange("p (r m) -> p r m", r=NCORES),
                            mhT_r[:, :, kt * MB : (kt + 1) * MB],
                            ACTF.Relu,
                            bias=mcp[:, kt * SH + i : kt * SH + i + 1],
                        )
                        scr = wpool.tile([128, JP], BF16, tag="scr")
                        b_inst = nc.vector.scalar_tensor_tensor(
                            out=scr[:],
                            in0=t_t[:],
                            scalar=0.0,
                            in1=blob[:, i * JP : (i + 1) * JP],
                            op0=ALU.add,
                            op1=ALU.mult,
                            accum_out=msgT[:, kt * SH + i : kt * SH + i + 1],
                        )
                        if i == SH - 4 and kt == 0:
                            late_b = b_inst

                # dense burst of tiny matmuls pinned to the tail of the
                # message loop: warms the PE HAM clock gate (cold 1.2GHz ->
                # 2.4GHz) right before the serial state-update matmul phase
                for _w in range(25):
                    pw = psA.tile([1, 1], F32, tag="psA")
                    wmm = nc.tensor.matmul(
                        pw[:], lhsT=wb[:, 0:1], rhs=wb[:, 0:1],
                        start=True, stop=True,
                    )
                    bass._add_dep_helper(
                        wmm.ins, late_b.ins, sync=True, reason="ham-warm"
                    )

                msgTb = wpool.tile([128, KT * SH], BF16, tag="msgTb")
                nc.vector.tensor_copy(msgTb[:], msgT[:])

                # ---- states = relu(g_in @ Wg + bg), T-layout ----
                rhs_tiles = (
                    [statesT[:, t * SH : (t + 1) * SH] for t in range(HT)]
                    + [
                        wb[:, INIT + t * SH : INIT + (t + 1) * SH]
                        for t in range(HT)
                    ]
                    + [msgTb[:, t * SH : (t + 1) * SH] for t in range(KT)]
                )
                # row-major states via swapped matmul (18 big-N matmuls
                # instead of 144 small ones); bg folded in via a ones-row
                strow = spool.tile([SH, H], BF16, tag="strow")
                if last:
                    fin = wpool.tile([SH, H], F32, tag="fin")
                for nt in range(2):
                    psr = psG.tile([SH, 512], F32, tag="psG")
                    for g in range(GT):
                        nc.tensor.matmul(
                            psr[:],
                            lhsT=rhs_tiles[g],
                            rhs=wb[
                                :, WG + g * H + nt * 512 : WG + g * H + nt * 512 + 512
                            ],
                            start=(g == 0),
                            stop=False,
                        )
                    nc.tensor.matmul(
                        psr[:],
                        lhsT=wb[0:1, ONES65 : ONES65 + SH],
                        rhs=wb[0:1, BGROW + nt * 512 : BGROW + nt * 512 + 512],
                        start=False,
                        stop=True,
                    )
                    nc.vector.tensor_scalar(
                        strow[:, nt * 512 : (nt + 1) * 512], psr[:], 0.0, None, ALU.max
                    )
                    if last:
                        nc.vector.tensor_scalar(
                            fin[0:SH, nt * 512 : (nt + 1) * 512],
                            psr[:],
                            0.0,
                            None,
                            ALU.max,
                        )
                # back to T-layout via PE transpose
                statesT_new = spool.tile([128, HT * SH], BF16, tag="stT")
                for ht in range(HT):
                    pst = psA.tile([128, SH], BF16, tag="psT")
                    nc.tensor.transpose(
                        pst[:],
                        strow[0:SH, ht * 128 : (ht + 1) * 128],
                        wb[0:SH, IDENT : IDENT + SH],
                    )
                    nc.vector.tensor_copy(
                        statesT_new[:, ht * SH : (ht + 1) * SH], pst[:]
                    )

                if last:
                    fin = wpool.tile([SH, H], F32, tag="fin")
                    # row-major fp32 states; bg folded in via a ones-row
                    for nt in range(2):
                        psf = psF.tile([SH, 512], F32, tag="psF")
                        for g in range(GT):
                            nc.tensor.matmul(
                                psf[:],
                                lhsT=rhs_tiles[g],
                                rhs=wb[
                                    :,
                                    WG + g * H + nt * 512 : WG + g * H + nt * 512 + 512,
                                ],
                                start=(g == 0),
                                stop=False,
                            )
                        nc.tensor.matmul(
                            psf[:],
                            lhsT=wb[0:1, ONES65 : ONES65 + SH],
                            rhs=wb[0:1, BGROW + nt * 512 : BGROW + nt * 512 + 512],
                            start=False,
                            stop=True,
                        )
                        nc.vector.tensor_scalar(
                            fin[0:SH, nt * 512 : (nt + 1) * 512],
                            psf[:],
                            0.0,
                            None,
                            ALU.max,
                        )

                    # logits from local row ANS_LOCAL (core 7's is the answer)
                    psl = psA.tile([1, 5], F32, tag="psA")
                    for ht in range(HT):
                        nc.tensor.matmul(
                            psl[:],
                            lhsT=statesT_new[
                                :, ht * SH + ANS_LOCAL : ht * SH + ANS_LOCAL + 1
                            ],
                            rhs=wb[:, WO + ht * 5 : WO + (ht + 1) * 5],
                            start=(ht == 0),
                            stop=(ht == HT - 1),
                        )
                    xlg = wpool.tile([1, 5], F32, tag="lg")
                    nc.vector.tensor_tensor(xlg[:], psl[:], bo_v, ALU.add)
                    mx = wpool.tile([1, 1], F32, tag="lg1")
                    nc.vector.tensor_reduce(
                        mx[:], xlg[:], mybir.AxisListType.X, ALU.max
                    )
                    nmx = wpool.tile([1, 1], F32, tag="lg2")
                    nc.vector.tensor_scalar(nmx[:], mx[:], -1.0, None, ALU.mult)
                    elg = wpool.tile([1, 5], F32, tag="lg3")
                    nc.scalar.activation(elg[:], xlg[:], ACTF.Exp, bias=nmx[:, 0:1])
                    ssum = wpool.tile([1, 1], F32, tag="lg4")
                    nc.vector.tensor_reduce(
                        ssum[:], elg[:], mybir.AxisListType.X, ALU.add
                    )
                    rcp = wpool.tile([1, 1], F32, tag="lg5")
                    nc.vector.reciprocal(rcp[:], ssum[:])
                    lgo = wpool.tile([1, 5], F32, tag="lg6")
                    nc.vector.tensor_scalar(
                        lgo[:], elg[:], rcp[:, 0:1], None, ALU.mult
                    )
                    nc.gpsimd.dma_start(p_out[:], fin[:])
                    nc.gpsimd.dma_start(p_lg[:], lgo[:])

                statesT = statesT_new

    nc.finalize()
    return nc


_NC_CACHE: dict[int, bass.Bass] = {}


def _f32_as_bf16pair(a):
    # reinterpret a float32 array as pairs of bf16-sized uint16 lanes
    a = np.ascontiguousarray(a.astype(np.float32))
    return a.view(np.uint16).view(BFNP)


def _prep_wblob(Wfc, Wfh, Wg, Wo, bf, bg, bo, initT):
    wb = np.zeros((128, NCOL), BFNP)
    wb[:, WFC : WFC + HT * MSG] = (
        Wfc.reshape(HT, 128, MSG).transpose(1, 0, 2).reshape(128, HT * MSG)
    ).astype(BFNP)
    wb[:, WFH : WFH + HT * MSG] = (
        Wfh.reshape(HT, 128, MSG).transpose(1, 0, 2).reshape(128, HT * MSG)
    ).astype(BFNP)
    wb[:, WG : WG + GT * H] = (
        Wg.reshape(GT, 128, H).transpose(1, 0, 2).reshape(128, GT * H)
    ).astype(BFNP)
    wb[:, WO : WO + HT * 5] = (
        Wo.reshape(HT, 128, 5).transpose(1, 0, 2).reshape(128, HT * 5)
    ).astype(BFNP)
    wb[:, INIT : INIT + HT * SH] = initT.astype(BFNP)
    wb[:, BF : BF + 2 * KT] = _f32_as_bf16pair(bf.reshape(KT, 128).T.copy())
    wb[:, BGC : BGC + 2 * HT] = _f32_as_bf16pair(bg.reshape(HT, 128).T.copy())
    wb[:, ONES128 : ONES128 + 128] = np.ones((128, 128), BFNP)
    wb[0, ONES65 : ONES65 + SH] = np.ones(SH, BFNP)
    wb[0, BGROW : BGROW + H] = bg.astype(BFNP)
    wb[0, BO : BO + 10] = _f32_as_bf16pair(bo.reshape(1, 5))[0]
    wb[:, IDENT : IDENT + 128] = np.eye(128, dtype=np.float32).astype(BFNP)
    return wb


def kernel(pooled_output, super_node, mask, Wf, bf, Wg, bg, Wo, bo, n_steps):
    n_steps = int(n_steps)
    pooled_output = np.asarray(pooled_output, np.float32)
    super_node = np.asarray(super_node, np.float32)
    mask = np.asarray(mask, np.float32)
    Wf = np.asarray(Wf, np.float32)
    bf = np.asarray(bf, np.float32)
    Wg = np.asarray(Wg, np.float32)
    bg = np.asarray(bg, np.float32)
    Wo = np.asarray(Wo, np.float32)
    bo = np.asarray(bo, np.float32)

    initial = np.concatenate([super_node, pooled_output], axis=0)  # (513, H)
    init_pad = np.zeros((PAD, H), np.float32)
    init_pad[:NP1] = initial
    mask2 = mask.reshape(NP1, NP1)
    mask_pad = np.zeros((PAD, PAD), np.float32)
    mask_pad[:NP1, :NP1] = mask2

    Wfc, Wfh = np.ascontiguousarray(Wf[:H]), np.ascontiguousarray(Wf[H:])

    in_maps = []
    for r in range(NCORES):
        sl = slice(r * SH, (r + 1) * SH)
        initT_r = np.ascontiguousarray(init_pad[sl].T)  # (H, 65)
        initT_r = (
            initT_r.reshape(HT, 128, SH).transpose(1, 0, 2).reshape(128, HT * SH)
        )  # [p, t*65+i]
        wb = _prep_wblob(Wfc, Wfh, Wg, Wo, bf, bg, bo, initT_r)
        # mask rows in blob j-order: col (rr*66 + m) = mask_pad[gi, 65*rr+m]
        mrows = np.zeros((SH, NCORES * MB), np.float32)
        msk = mask_pad[sl, :]  # (65, 520)
        for rr in range(NCORES):
            mrows[:, rr * MB : rr * MB + SH] = msk[:, rr * SH : (rr + 1) * SH]
        msb = np.zeros((MS_ROWS, MS_COLS), BFNP)
        JPc = NCORES * MB
        for i in range(SH):
            msb[32 * (i % 3), (i // 3) * JPc : (i // 3 + 1) * JPc] = mrows[i].astype(
                BFNP
            )
        in_maps.append(dict(wblob=wb, maskb=msb))

    if n_steps not in _NC_CACHE:
        _NC_CACHE[n_steps] = build_nc(n_steps)
    nc = _NC_CACHE[n_steps]

    res = run_bass_kernel_spmd(nc, in_maps, core_ids=list(range(NCORES)))
    global LAST_RESULT
    LAST_RESULT = res
    results = res.results
    states = np.concatenate(
        [np.asarray(results[r]["outb"], np.float32) for r in range(NCORES)],
        axis=0,
    )[:NP1]
    logits = np.asarray(results[7]["out_logits"], np.float32).copy()
    return logits, states


LAST_RESULT = None
